# revision 53
# baseline (speedup 1.0000x reference)
"""GQA attention kernel for Trainium2, sharded over 8 NeuronCores.

Sharding: core c = b*4 + g handles batch b and GQA group g (4 query heads
+ 1 KV head). Wq/Wk/Wv column-sharded per group, Wo row-sharded; the host
sums the 4 per-group partial outputs per batch.

Device layout tricks:
  - x is passed transposed (xT [D, S]) so Q^T/K^T project directly into
    [head_dim, S] layout (head_dim on partitions).
  - V is projected as V^T [dh, S] with N=512 matmuls (cheap), then PE
    transposes produce the [S, dh] tiles PV needs as stationary operand.
  - Q/K head dims are de-interleaved host-side (even dims then odd dims)
    by permuting Wq/Wk columns, making RoPE a half-tile multiply/add.
    Scores are invariant to a shared permutation of Q/K dims.
  - RoPE runs on the DVE in a packed [64, 2, 512] bf16 layout (both
    halves on partitions 0:64, the half-swap becomes a free-dim offset)
    with duplicated cos / sign-folded sin constants: 5 tensor ops per
    chunk, all-SBUF all-bf16 so the DVE's fast mode engages.
  - Attention computes scoresT [key, query] so softmax exp output is
    directly the lhs^T operand ("P^T") for the P@V matmul.
  - 1/sqrt(dh) is folded into the exp activation's scale; the causal mask
    is a 0/1 multiply (GpSimd) on the diagonal-straddling blocks per
    q-block, truncated to the prefix of columns that can contain masked
    entries; the inner loop is software-pipelined by one k-chunk so the
    exp+mask latency hides under the next score matmul.
  - softmax denominator: accumulated on the PE via ones-matmuls into a
    [1,512] PSUM region, then reciprocal_approx_fast + a K=1 ones-matmul
    broadcasts 1/l across partitions for the normalization multiply.
  - Everything on the PE is bf16 (fp32 matmuls run at 1/4 rate).
  - The per-s-chunk projection, the attention q-block it unlocks, and
    the previous q-block's Wo projection are interleaved so every
    engine's latency hides under PE matmul streams.
"""

import sys

if "/opt/trn_rl_repo" not in sys.path:
    sys.path.insert(0, "/opt/trn_rl_repo")

import numpy as np
import ml_dtypes

import concourse.bass as bass
import concourse.bacc as bacc
import concourse.tile as tile
from concourse import mybir
from concourse.bass_utils import run_bass_kernel_spmd
from concourse.masks import make_identity

B = 2
S = 2048
D = 2048
N_HEADS = 16
N_KV = 4
DH = 128
NH = 4  # query heads per core
N_CORES = 8

INV_SQRT_DH = 1.0 / np.sqrt(DH)
F32 = mybir.dt.float32
BF16 = mybir.dt.bfloat16


def build_program(s=S, d=D):
    """Per-core program: 4 query heads + 1 KV head of causal GQA."""
    kc_n = d // 128       # contraction chunks
    sc = 512              # projection s-chunk / q-block
    nsc = s // sc
    qb_n = s // 512

    nc = bacc.Bacc("TRN2", target_bir_lowering=False, debug=False,
                   num_devices=N_CORES)
    xT = nc.declare_dram_parameter("xT", [d, s], BF16, isOutput=False)
    wq = nc.declare_dram_parameter("wq", [d, NH * DH], BF16, isOutput=False)
    wkv = nc.declare_dram_parameter("wkv", [d, 2 * DH], BF16, isOutput=False)
    wo = nc.declare_dram_parameter("wo", [NH * DH, d], BF16, isOutput=False)
    cos2 = nc.declare_dram_parameter("cos2", [64, 2, s], BF16, isOutput=False)
    sin2 = nc.declare_dram_parameter("sin2", [64, 2, s], BF16, isOutput=False)
    maskb = nc.declare_dram_parameter("maskb", [128, 896], BF16, isOutput=False)
    out_p = nc.declare_dram_parameter("out_p", [s, d], BF16, isOutput=True)

    with tile.TileContext(nc) as tc:
        with (
            tc.tile_pool(name="const", bufs=1) as cpool,
            tc.tile_pool(name="xp", bufs=1) as xpool,
            tc.tile_pool(name="act", bufs=1) as apool,
            tc.tile_pool(name="tmp", bufs=1) as tpool,
            tc.tile_pool(name="psum", bufs=1, space="PSUM") as pp,
        ):
            # ---- constants (wo is DMA'd later; it is needed only by the
            #      first wo_block, which runs after the second att block).
            #      wq/wkv are split into chunk DMAs so they spread across
            #      DMA queues and the first projection matmuls start early.
            wq_sb = cpool.tile([128, kc_n, NH * DH], BF16, tag="wq")
            wq_r = wq.rearrange("(n p) m -> p n m", p=128)
            wkv_sb = cpool.tile([128, kc_n, 2 * DH], BF16, tag="wkv")
            wkv_r = wkv.rearrange("(n p) m -> p n m", p=128)
            # The first projection group is K/V: their weights (wkv) and the
            # first x chunks go out first so compute starts ~3us in, with wq
            # following (q0's matmuls interleave per-kc and catch up).
            xt0 = []
            for kc in range(kc_n):
                if kc % 4 == 0:
                    nc.sync.dma_start(wkv_sb[:, kc:kc + 4, :],
                                      wkv_r[:, kc:kc + 4, :])
                t = xpool.tile([128, sc], BF16, tag=f"x{kc}", bufs=2,
                               name=f"xt0_{kc}")
                nc.sync.dma_start(t[:], xT[kc * 128:(kc + 1) * 128, 0:sc])
                xt0.append(t)
            for j in range(8):
                nc.sync.dma_start(wq_sb[:, 2 * j:2 * j + 2, :],
                                  wq_r[:, 2 * j:2 * j + 2, :])
            cos_sb = cpool.tile([64, 2, s], BF16, tag="cos")
            nc.sync.dma_start(cos_sb[:], cos2[:])
            sin_sb = cpool.tile([64, 2, s], BF16, tag="sin")
            nc.sync.dma_start(sin_sb[:], sin2[:])
            mask_sb = cpool.tile([128, 896], BF16, tag="mask")
            nc.sync.dma_start(mask_sb[:], maskb[:])
            ones_col = cpool.tile([128, 1], BF16, tag="ones_col")
            nc.vector.memset(ones_col[:], 1.0)
            ones_row = cpool.tile([1, 128], BF16, tag="ones_row")
            nc.vector.memset(ones_row[:], 1.0)
            ident = cpool.tile([128, 128], BF16, tag="ident")
            make_identity(nc, ident[:])
            wo_sb = cpool.tile([128, NH, d], BF16, tag="wo")

            # ---- persistent activations ----
            ktr = apool.tile([128, s], BF16, tag="ktr")
            qtr = {}   # (h, qb) -> tile
            v_sb = {}  # sc_i -> [128, 4*128] tile (4 key chunks)
            otr = {}   # (h, qb) -> tile

            def rope(dsl, src_psum, sc_i):
                """dsl ([128, sc] slice) = rope(src) with de-interleaved halves.

                src rows 0:64 = even dims (a), 64:128 = odd dims (b).
                re = a*c - b*s -> rows 0:64 ; ro = a*s + b*c -> rows 64:128.
                Packed: braw2 [64, 2, sc] = [a | b] on partitions 0:64; the
                half-swap is a free-dim offset; sin2 carries [-s | +s].
                """
                c = cos_sb[:, :, sc_i * sc:(sc_i + 1) * sc]
                sn = sin_sb[:, :, sc_i * sc:(sc_i + 1) * sc]
                braw2 = tpool.tile([64, 2, sc], BF16, tag="braw", bufs=2)
                nc.scalar.copy(braw2[:, 0, :], src_psum[0:64, :])
                nc.scalar.copy(braw2[:, 1, :], src_psum[64:128, :])
                t1 = tpool.tile([64, 2, sc], BF16, tag="t1", bufs=2)
                t2 = tpool.tile([64, 2, sc], BF16, tag="t2", bufs=2)
                nc.vector.tensor_mul(t1[:], braw2[:], c)
                nc.vector.tensor_mul(t2[:, 0, :], braw2[:, 1, :], sn[:, 0, :])
                nc.vector.tensor_mul(t2[:, 1, :], braw2[:, 0, :], sn[:, 1, :])
                nc.vector.tensor_add(dsl[0:64, :], t1[:, 0, :], t2[:, 0, :])
                nc.vector.tensor_add(dsl[64:128, :], t1[:, 1, :], t2[:, 1, :])

            def emit_xt(sc_i):
                xt = []
                for kc in range(kc_n):
                    t = xpool.tile([128, sc], BF16, tag=f"x{kc}", bufs=2,
                                   name=f"xt{sc_i}_{kc}")
                    nc.sync.dma_start(
                        t[:],
                        xT[kc * 128:(kc + 1) * 128, sc_i * sc:(sc_i + 1) * sc]
                    )
                    xt.append(t)
                return xt

            def proj_block(sc_i, xt):
                # K (4) and V^T (5) first so the att block's first score
                # matmuls never wait on the K rope / V transpose. Groups of 2
                # so the pp PSUM ring needs only 2 banks (freeing one for the
                # double-banked otp).
                for grp in ([4, 5], [0, 1], [2, 3]):
                    psums = {}
                    for hh in grp:
                        psums[hh] = pp.tile([128, sc], F32, tag="pp", bufs=2,
                                            name=f"pj{sc_i}_{hh}")
                    for kc in range(kc_n):
                        for hh in grp:
                            if hh < NH:
                                lhsT = wq_sb[:, kc, hh * DH:(hh + 1) * DH]
                            elif hh == NH:
                                lhsT = wkv_sb[:, kc, 0:DH]
                            else:
                                lhsT = wkv_sb[:, kc, DH:2 * DH]
                            nc.tensor.matmul(
                                psums[hh][:], lhsT, xt[kc][:],
                                start=(kc == 0), stop=(kc == kc_n - 1),
                            )
                    for hh in grp:
                        if hh < NH:
                            qtr[(hh, sc_i)] = apool.tile(
                                [128, 512], BF16, tag=f"qtr{hh}", bufs=4,
                                name=f"qtr{hh}_{sc_i}")
                            rope(qtr[(hh, sc_i)][:, :], psums[hh], sc_i)
                        elif hh == NH:
                            rope(ktr[:, sc_i * sc:(sc_i + 1) * sc], psums[hh], sc_i)
                        else:
                            # V^T [dh, sc] -> bf16 SBUF, then PE-transpose each
                            # 128-wide block into the natural [S, dh] v tiles
                            # (4 transposes packed into one [128,512] PSUM tile).
                            vt_sb = tpool.tile([128, sc], BF16, tag="vt", bufs=2,
                                               name=f"vt{sc_i}")
                            nc.scalar.copy(vt_sb[:], psums[hh][:])
                            vp4 = pp.tile([128, sc], BF16, tag="wp", bufs=2,
                                          name=f"vtp{sc_i}")
                            for stl in range(sc // 128):
                                nc.tensor.transpose(
                                    vp4[:, stl * 128:(stl + 1) * 128],
                                    vt_sb[:, stl * 128:(stl + 1) * 128],
                                    ident[:])
                            v_sb[sc_i] = apool.tile([128, sc], BF16,
                                                    tag=f"v{sc_i}", name=f"v{sc_i}")
                            nc.scalar.copy(v_sb[sc_i][:], vp4[:])

            def att_block(qb):
                # Normalization tail of head h-1 (rlb matmul -> rlb_sb copy
                # -> otr multiply) is deferred until after head h's kc loop:
                # its recip chain completes under a full head of PE matmuls,
                # so the rlb matmul never stalls the PE. otp is double-banked
                # so head h's PV accumulation doesn't wait on head h-1's
                # deferred otr read.
                pend = {}

                def norm_tail(j):
                    rlb = pp.tile([128, 512], F32, tag="lr", bufs=2,
                                  name=f"rlb{j}_{qb}")
                    nc.tensor.matmul(rlb[:], ones_row[:], pend[j][0][:],
                                     start=True, stop=True)
                    rlb_sb = tpool.tile([128, 512], BF16, tag="rlbs", bufs=2,
                                        name=f"rlbs{j}_{qb}")
                    nc.scalar.copy(rlb_sb[:], rlb[:])
                    otr[(j, qb)] = apool.tile([128, 512], BF16, tag=f"otr{j}",
                                              bufs=2, name=f"otr{j}_{qb}")
                    nc.vector.tensor_mul(otr[(j, qb)][:], pend[j][1][:],
                                         rlb_sb[:])

                for h in range(NH):
                    nkc = 4 * (qb + 1)
                    otp = pp.tile([128, 512], F32, tag="ot", bufs=2,
                                  name=f"otp{h}_{qb}")
                    lp = pp.tile([1, 512], F32, tag="lr", bufs=2,
                                 name=f"lp{h}_{qb}")

                    def tail(kc):
                        # lsum + PV matmuls for chunk kc (issued one chunk
                        # late so exp+mask latency hides under the next
                        # score matmul)
                        nc.tensor.matmul(
                            lp[:], ones_col[:], pts[kc][:],
                            start=(kc == 0), stop=(kc == nkc - 1),
                        )
                        nc.tensor.matmul(
                            otp[:],
                            v_sb[kc // 4][:, (kc % 4) * 128:(kc % 4 + 1) * 128],
                            pts[kc][:],
                            start=(kc == 0), stop=(kc == nkc - 1),
                        )

                    pts = {}
                    for kc in range(nkc):
                        scp = pp.tile([128, 512], F32, tag="pp", bufs=2,
                                      name=f"scp{h}_{qb}_{kc}")
                        nc.tensor.matmul(
                            scp[:], ktr[:, kc * 128:(kc + 1) * 128],
                            qtr[(h, qb)][:], start=True, stop=True,
                        )
                        pts[kc] = tpool.tile([128, 512], BF16, tag="pt", bufs=4,
                                             name=f"pt{h}_{qb}_{kc}")
                        nc.scalar.activation(
                            pts[kc][:], scp[:], mybir.ActivationFunctionType.Exp,
                            scale=float(INV_SQRT_DH),
                        )
                        if kc >= 4 * qb:  # diagonal-straddling: zero disallowed.
                            # Only cols 0:w can contain masked entries.
                            m = kc - 4 * qb
                            stt = 384 - m * 128
                            w = (m + 1) * 128
                            nc.gpsimd.tensor_mul(pts[kc][:, 0:w],
                                                 pts[kc][:, 0:w],
                                                 mask_sb[:, stt:stt + w])
                        if kc > 0:
                            tail(kc - 1)
                    tail(nkc - 1)

                    if h > 0:
                        norm_tail(h - 1)
                    rl = tpool.tile([1, 512], F32, tag="rl", bufs=2,
                                    name=f"rl{h}_{qb}")
                    nc.vector.reciprocal_approx_fast(rl[:], lp[:])
                    rl_bf = tpool.tile([1, 512], BF16, tag="rlb16", bufs=2,
                                       name=f"rlbf{h}_{qb}")
                    nc.vector.tensor_copy(rl_bf[:], rl[:])
                    pend[h] = (rl_bf, otp)
                norm_tail(NH - 1)

            def wo_block(qb, final=False):
                for stl in range(4):
                    st = 4 * qb + stl
                    for dm in range(d // 512):
                        wop = pp.tile([128, 512], F32, tag="wp", bufs=2,
                                      name=f"wop{st}_{dm}")
                        for h in range(NH):
                            nc.tensor.matmul(
                                wop[:],
                                otr[(h, qb)][:, stl * 128:(stl + 1) * 128],
                                wo_sb[:, h, dm * 512:(dm + 1) * 512],
                                start=(h == 0), stop=(h == NH - 1),
                            )
                        osb = tpool.tile([128, 512], BF16, tag="osb", bufs=3,
                                         name=f"osb{st}_{dm}")
                        if final and dm % 2 == 1:
                            # the last block has no att block to hide behind;
                            # split the PSUM drains across ACT and DVE
                            nc.scalar.copy(osb[:], wop[:])
                        else:
                            nc.vector.tensor_copy(osb[:], wop[:])
                        nc.sync.dma_start(
                            out_p[st * 128:(st + 1) * 128, dm * 512:(dm + 1) * 512],
                            osb[:],
                        )

            xt_next = xt0
            for sc_i in range(nsc):
                proj_block(sc_i, xt_next)
                if sc_i == 0:
                    nc.sync.dma_start(
                        wo_sb[:], wo.rearrange("(n p) m -> p n m", p=128))
                if sc_i + 1 < nsc:
                    # prefetch-emit the next chunk's x DMAs so they sit ahead
                    # of the att/wo blocks' output traffic in the queues
                    xt_next = emit_xt(sc_i + 1)
                att_block(sc_i)
                if sc_i >= 1:
                    wo_block(sc_i - 1)
            wo_block(qb_n - 1, final=True)

    nc.compile()
    return nc


_PROGRAM = None


def _get_program():
    global _PROGRAM
    if _PROGRAM is None:
        _PROGRAM = build_program()
    return _PROGRAM


_DEINT = np.concatenate([np.arange(0, DH, 2), np.arange(1, DH, 2)])


def make_in_maps(x, rope_cos, rope_sin, Wq, Wk, Wv, Wo, s=S):
    cosT = rope_cos[:s].T.astype(np.float32)
    sinT = rope_sin[:s].T.astype(np.float32)
    cos2 = np.ascontiguousarray(
        np.stack([cosT, cosT], axis=1).astype(ml_dtypes.bfloat16))
    sin2 = np.ascontiguousarray(
        np.stack([-sinT, sinT], axis=1).astype(ml_dtypes.bfloat16))
    kp = np.arange(128)[:, None]
    cc = np.arange(896)[None, :]
    maskb = (cc >= kp + 384).astype(ml_dtypes.bfloat16)
    in_maps = []
    for c in range(N_CORES):
        b, g = divmod(c, 4)
        xTc = np.ascontiguousarray(x[b].T.astype(ml_dtypes.bfloat16))
        wq_cols = [
            Wq[:, (g * NH + j) * DH:(g * NH + j + 1) * DH][:, _DEINT]
            for j in range(NH)
        ]
        wq_c = np.ascontiguousarray(np.concatenate(wq_cols, axis=1).astype(ml_dtypes.bfloat16))
        wk_c = Wk[:, g * DH:(g + 1) * DH][:, _DEINT]
        wv_c = Wv[:, g * DH:(g + 1) * DH]
        wkv_c = np.ascontiguousarray(
            np.concatenate([wk_c, wv_c], axis=1).astype(ml_dtypes.bfloat16))
        wo_c = np.ascontiguousarray(
            Wo[g * NH * DH:(g + 1) * NH * DH, :].astype(ml_dtypes.bfloat16))
        in_maps.append({
            "xT": xTc, "wq": wq_c, "wkv": wkv_c, "wo": wo_c,
            "cos2": cos2, "sin2": sin2, "maskb": maskb,
        })
    return in_maps


def kernel(x, rope_cos, rope_sin, Wq, Wk, Wv, Wo):
    nc = _get_program()
    in_maps = make_in_maps(x, rope_cos, rope_sin, Wq, Wk, Wv, Wo)
    res = run_bass_kernel_spmd(nc, in_maps, list(range(N_CORES)))
    out = np.zeros((B, S, D), dtype=np.float32)
    for c in range(N_CORES):
        b, g = divmod(c, 4)
        out[b] += res.results[c]["out_p"].astype(np.float32)
    return out


# revision 54
# speedup vs baseline: 1.0534x; 1.0534x over previous
"""GQA attention kernel for Trainium2, sharded over 8 NeuronCores.

Sharding: core c = b*4 + g handles batch b and GQA group g (4 query heads
+ 1 KV head). Wq/Wk/Wv column-sharded per group, Wo row-sharded; the host
sums the 4 per-group partial outputs per batch.

Device layout tricks:
  - x is passed transposed (xT [D, S]) so Q^T/K^T project directly into
    [head_dim, S] layout (head_dim on partitions).
  - V is projected as V^T [dh, S] with N=512 matmuls (cheap), then PE
    transposes produce the [S, dh] tiles PV needs as stationary operand.
  - Q/K head dims are de-interleaved host-side (even dims then odd dims)
    by permuting Wq/Wk columns, making RoPE a half-tile multiply/add.
    Scores are invariant to a shared permutation of Q/K dims.
  - RoPE runs on the DVE in a packed [64, 2, 512] bf16 layout (both
    halves on partitions 0:64, the half-swap becomes a free-dim offset)
    with duplicated cos / sign-folded sin constants: 5 tensor ops per
    chunk, all-SBUF all-bf16 so the DVE's fast mode engages.
  - Attention computes scoresT [key, query] so softmax exp output is
    directly the lhs^T operand ("P^T") for the P@V matmul.
  - 1/sqrt(dh) is folded into the exp activation's scale; the causal mask
    is a 0/1 multiply (GpSimd) on the diagonal-straddling blocks per
    q-block, truncated to the prefix of columns that can contain masked
    entries; the inner loop is software-pipelined by one k-chunk so the
    exp+mask latency hides under the next score matmul.
  - softmax denominator: accumulated on the PE via ones-matmuls into a
    [1,512] PSUM region, then reciprocal_approx_fast + a K=1 ones-matmul
    broadcasts 1/l across partitions for the normalization multiply.
  - Everything on the PE is bf16 (fp32 matmuls run at 1/4 rate).
  - The per-s-chunk projection, the attention q-block it unlocks, and
    the previous q-block's Wo projection are interleaved so every
    engine's latency hides under PE matmul streams.
"""

import sys

if "/opt/trn_rl_repo" not in sys.path:
    sys.path.insert(0, "/opt/trn_rl_repo")

import numpy as np
import ml_dtypes

import concourse.bass as bass
import concourse.bacc as bacc
import concourse.tile as tile
from concourse import mybir
from concourse.bass_utils import run_bass_kernel_spmd
from concourse.masks import make_identity

B = 2
S = 2048
D = 2048
N_HEADS = 16
N_KV = 4
DH = 128
NH = 4  # query heads per core
N_CORES = 8

INV_SQRT_DH = 1.0 / np.sqrt(DH)
F32 = mybir.dt.float32
BF16 = mybir.dt.bfloat16


def build_program(s=S, d=D):
    """Per-core program: 4 query heads + 1 KV head of causal GQA."""
    kc_n = d // 128       # contraction chunks
    sc = 512              # projection s-chunk / q-block
    nsc = s // sc
    qb_n = s // 512

    nc = bacc.Bacc("TRN2", target_bir_lowering=False, debug=False,
                   num_devices=N_CORES)
    xT = nc.declare_dram_parameter("xT", [d, s], BF16, isOutput=False)
    wq = nc.declare_dram_parameter("wq", [d, NH * DH], BF16, isOutput=False)
    wkv = nc.declare_dram_parameter("wkv", [d, 2 * DH], BF16, isOutput=False)
    wo = nc.declare_dram_parameter("wo", [NH * DH, d], BF16, isOutput=False)
    cos2 = nc.declare_dram_parameter("cos2", [64, 2, s], BF16, isOutput=False)
    sin2 = nc.declare_dram_parameter("sin2", [64, 2, s], BF16, isOutput=False)
    maskb = nc.declare_dram_parameter("maskb", [128, 896], BF16, isOutput=False)
    out_p = nc.declare_dram_parameter("out_p", [s, d], BF16, isOutput=True)

    with tile.TileContext(nc) as tc:
        with (
            tc.tile_pool(name="const", bufs=1) as cpool,
            tc.tile_pool(name="xp", bufs=1) as xpool,
            tc.tile_pool(name="act", bufs=1) as apool,
            tc.tile_pool(name="tmp", bufs=1) as tpool,
            tc.tile_pool(name="psum", bufs=1, space="PSUM") as pp,
        ):
            # ---- constants (wo is DMA'd later; it is needed only by the
            #      first wo_block, which runs after the second att block).
            #      wq/wkv are split into chunk DMAs so they spread across
            #      DMA queues and the first projection matmuls start early.
            wq_sb = cpool.tile([128, kc_n, NH * DH], BF16, tag="wq")
            wq_r = wq.rearrange("(n p) m -> p n m", p=128)
            wkv_sb = cpool.tile([128, kc_n, 2 * DH], BF16, tag="wkv")
            wkv_r = wkv.rearrange("(n p) m -> p n m", p=128)
            # The first projection group is K/V: their weights (wkv) and the
            # first x chunks go out first so compute starts ~3us in, with wq
            # following (q0's matmuls interleave per-kc and catch up).
            xt0 = []
            for kc in range(kc_n):
                if kc % 4 == 0:
                    nc.sync.dma_start(wkv_sb[:, kc:kc + 4, :],
                                      wkv_r[:, kc:kc + 4, :])
                t = xpool.tile([128, sc], BF16, tag=f"x{kc}", bufs=2,
                               name=f"xt0_{kc}")
                nc.sync.dma_start(t[:], xT[kc * 128:(kc + 1) * 128, 0:sc])
                xt0.append(t)
            for j in range(8):
                nc.sync.dma_start(wq_sb[:, 2 * j:2 * j + 2, :],
                                  wq_r[:, 2 * j:2 * j + 2, :])
            cos_sb = cpool.tile([64, 2, s], BF16, tag="cos")
            nc.sync.dma_start(cos_sb[:], cos2[:])
            sin_sb = cpool.tile([64, 2, s], BF16, tag="sin")
            nc.sync.dma_start(sin_sb[:], sin2[:])
            mask_sb = cpool.tile([128, 896], BF16, tag="mask")
            nc.sync.dma_start(mask_sb[:], maskb[:])
            ones_col = cpool.tile([128, 1], BF16, tag="ones_col")
            nc.vector.memset(ones_col[:], 1.0)
            ones_row = cpool.tile([1, 128], BF16, tag="ones_row")
            nc.vector.memset(ones_row[:], 1.0)
            ident = cpool.tile([128, 128], BF16, tag="ident")
            make_identity(nc, ident[:])
            wo_sb = cpool.tile([128, NH, d], BF16, tag="wo")

            # ---- persistent activations ----
            ktr = apool.tile([128, s], BF16, tag="ktr")
            qtr = {}   # (h, qb) -> tile
            v_sb = {}  # sc_i -> [128, 4*128] tile (4 key chunks)
            otr = {}   # (h, qb) -> tile

            def rope(dsl, src_psum, sc_i):
                """dsl ([128, sc] slice) = rope(src) with de-interleaved halves.

                src rows 0:64 = even dims (a), 64:128 = odd dims (b).
                re = a*c - b*s -> rows 0:64 ; ro = a*s + b*c -> rows 64:128.
                Packed: braw2 [64, 2, sc] = [a | b] on partitions 0:64; the
                half-swap is a free-dim offset; sin2 carries [-s | +s].
                """
                c = cos_sb[:, :, sc_i * sc:(sc_i + 1) * sc]
                sn = sin_sb[:, :, sc_i * sc:(sc_i + 1) * sc]
                braw2 = tpool.tile([64, 2, sc], BF16, tag="braw", bufs=2)
                nc.scalar.copy(braw2[:, 0, :], src_psum[0:64, :])
                nc.scalar.copy(braw2[:, 1, :], src_psum[64:128, :])
                t1 = tpool.tile([64, 2, sc], BF16, tag="t1", bufs=2)
                t2 = tpool.tile([64, 2, sc], BF16, tag="t2", bufs=2)
                nc.vector.tensor_mul(t1[:], braw2[:], c)
                nc.vector.tensor_mul(t2[:, 0, :], braw2[:, 1, :], sn[:, 0, :])
                nc.vector.tensor_mul(t2[:, 1, :], braw2[:, 0, :], sn[:, 1, :])
                nc.vector.tensor_add(dsl[0:64, :], t1[:, 0, :], t2[:, 0, :])
                nc.vector.tensor_add(dsl[64:128, :], t1[:, 1, :], t2[:, 1, :])

            def emit_xt(sc_i):
                xt = []
                for kc in range(kc_n):
                    t = xpool.tile([128, sc], BF16, tag=f"x{kc}", bufs=2,
                                   name=f"xt{sc_i}_{kc}")
                    nc.sync.dma_start(
                        t[:],
                        xT[kc * 128:(kc + 1) * 128, sc_i * sc:(sc_i + 1) * sc]
                    )
                    xt.append(t)
                return xt

            def proj_block(sc_i, xt):
                # K (4) and V^T (5) first so the att block's first score
                # matmuls never wait on the K rope / V transpose. Groups of 2
                # so the pp PSUM ring needs only 2 banks (freeing one for the
                # double-banked otp).
                for grp in ([4, 5], [0, 1], [2, 3]):
                    psums = {}
                    for hh in grp:
                        psums[hh] = pp.tile([128, sc], F32, tag="pp", bufs=2,
                                            name=f"pj{sc_i}_{hh}")
                    for kc in range(kc_n):
                        for hh in grp:
                            if hh < NH:
                                lhsT = wq_sb[:, kc, hh * DH:(hh + 1) * DH]
                            elif hh == NH:
                                lhsT = wkv_sb[:, kc, 0:DH]
                            else:
                                lhsT = wkv_sb[:, kc, DH:2 * DH]
                            nc.tensor.matmul(
                                psums[hh][:], lhsT, xt[kc][:],
                                start=(kc == 0), stop=(kc == kc_n - 1),
                            )
                    for hh in grp:
                        if hh < NH:
                            qtr[(hh, sc_i)] = apool.tile(
                                [128, 512], BF16, tag=f"qtr{hh}", bufs=4,
                                name=f"qtr{hh}_{sc_i}")
                            rope(qtr[(hh, sc_i)][:, :], psums[hh], sc_i)
                        elif hh == NH:
                            rope(ktr[:, sc_i * sc:(sc_i + 1) * sc], psums[hh], sc_i)
                        else:
                            # V^T [dh, sc] -> bf16 SBUF, then PE-transpose each
                            # 128-wide block into the natural [S, dh] v tiles
                            # (4 transposes packed into one [128,512] PSUM tile).
                            vt_sb = tpool.tile([128, sc], BF16, tag="vt", bufs=2,
                                               name=f"vt{sc_i}")
                            nc.scalar.copy(vt_sb[:], psums[hh][:])
                            vp4 = pp.tile([128, sc], BF16, tag="wp", bufs=2,
                                          name=f"vtp{sc_i}")
                            for stl in range(sc // 128):
                                nc.tensor.transpose(
                                    vp4[:, stl * 128:(stl + 1) * 128],
                                    vt_sb[:, stl * 128:(stl + 1) * 128],
                                    ident[:])
                            v_sb[sc_i] = apool.tile([128, sc], BF16,
                                                    tag=f"v{sc_i}", name=f"v{sc_i}")
                            nc.scalar.copy(v_sb[sc_i][:], vp4[:])

            def att_block(qb):
                # Normalization tail of head h-1 (rlb matmul -> rlb_sb copy
                # -> otr multiply) is deferred until after head h's kc loop:
                # its recip chain completes under a full head of PE matmuls,
                # so the rlb matmul never stalls the PE. otp is double-banked
                # so head h's PV accumulation doesn't wait on head h-1's
                # deferred otr read.
                pend = {}

                def norm_tail(j):
                    rlb = pp.tile([128, 512], F32, tag="lr", bufs=2,
                                  name=f"rlb{j}_{qb}")
                    nc.tensor.matmul(rlb[:], ones_row[:], pend[j][0][:],
                                     start=True, stop=True)
                    rlb_sb = tpool.tile([128, 512], BF16, tag="rlbs", bufs=2,
                                        name=f"rlbs{j}_{qb}")
                    nc.scalar.copy(rlb_sb[:], rlb[:])
                    otr[(j, qb)] = apool.tile([128, 512], BF16, tag=f"otr{j}",
                                              bufs=2, name=f"otr{j}_{qb}")
                    nc.vector.tensor_mul(otr[(j, qb)][:], pend[j][1][:],
                                         rlb_sb[:])

                for h in range(NH):
                    nkc = 4 * (qb + 1)
                    otp = pp.tile([128, 512], F32, tag="ot", bufs=2,
                                  name=f"otp{h}_{qb}")
                    lp = pp.tile([1, 512], F32, tag="lr", bufs=2,
                                 name=f"lp{h}_{qb}")

                    def tail(kc):
                        # PV matmul for chunk kc (issued one chunk late so
                        # exp+mask latency hides under the next score matmul)
                        nc.tensor.matmul(
                            otp[:],
                            v_sb[kc // 4][:, (kc % 4) * 128:(kc % 4 + 1) * 128],
                            pts[kc][:],
                            start=(kc == 0), stop=(kc == nkc - 1),
                        )

                    def tail_l(pr):
                        # denominator ones-matmul per chunk PAIR: adjacent exp
                        # chunks are pre-summed on the DVE (bf16 fast mode),
                        # halving the PE's lsum matmul count
                        nc.tensor.matmul(
                            lp[:], ones_col[:], ptsum[pr][:],
                            start=(pr == 0), stop=(pr == nkc // 2 - 1),
                        )

                    pts = {}
                    ptsum = {}
                    for kc in range(nkc):
                        scp = pp.tile([128, 512], F32, tag="pp", bufs=2,
                                      name=f"scp{h}_{qb}_{kc}")
                        nc.tensor.matmul(
                            scp[:], ktr[:, kc * 128:(kc + 1) * 128],
                            qtr[(h, qb)][:], start=True, stop=True,
                        )
                        pts[kc] = tpool.tile([128, 512], BF16, tag="pt", bufs=4,
                                             name=f"pt{h}_{qb}_{kc}")
                        nc.scalar.activation(
                            pts[kc][:], scp[:], mybir.ActivationFunctionType.Exp,
                            scale=float(INV_SQRT_DH),
                        )
                        if kc >= 4 * qb:  # diagonal-straddling: zero disallowed.
                            # Only cols 0:w can contain masked entries.
                            m = kc - 4 * qb
                            stt = 384 - m * 128
                            w = (m + 1) * 128
                            nc.gpsimd.tensor_mul(pts[kc][:, 0:w],
                                                 pts[kc][:, 0:w],
                                                 mask_sb[:, stt:stt + w])
                        if kc % 2 == 1:
                            ptsum[kc // 2] = tpool.tile(
                                [128, 512], BF16, tag="pts2", bufs=3,
                                name=f"ps{h}_{qb}_{kc // 2}")
                            nc.vector.tensor_add(ptsum[kc // 2][:],
                                                 pts[kc - 1][:], pts[kc][:])
                        if kc > 0:
                            tail(kc - 1)
                        if kc % 2 == 1 and kc // 2 > 0:
                            tail_l(kc // 2 - 1)
                    tail(nkc - 1)
                    tail_l(nkc // 2 - 1)

                    if h > 0:
                        norm_tail(h - 1)
                    rl = tpool.tile([1, 512], F32, tag="rl", bufs=2,
                                    name=f"rl{h}_{qb}")
                    nc.vector.reciprocal_approx_fast(rl[:], lp[:])
                    rl_bf = tpool.tile([1, 512], BF16, tag="rlb16", bufs=2,
                                       name=f"rlbf{h}_{qb}")
                    nc.vector.tensor_copy(rl_bf[:], rl[:])
                    pend[h] = (rl_bf, otp)
                norm_tail(NH - 1)

            def wo_block(qb, final=False):
                for stl in range(4):
                    st = 4 * qb + stl
                    for dm in range(d // 512):
                        wop = pp.tile([128, 512], F32, tag="wp", bufs=2,
                                      name=f"wop{st}_{dm}")
                        for h in range(NH):
                            nc.tensor.matmul(
                                wop[:],
                                otr[(h, qb)][:, stl * 128:(stl + 1) * 128],
                                wo_sb[:, h, dm * 512:(dm + 1) * 512],
                                start=(h == 0), stop=(h == NH - 1),
                            )
                        osb = tpool.tile([128, 512], BF16, tag="osb", bufs=3,
                                         name=f"osb{st}_{dm}")
                        if final and dm % 2 == 1:
                            # the last block has no att block to hide behind;
                            # split the PSUM drains across ACT and DVE
                            nc.scalar.copy(osb[:], wop[:])
                        else:
                            nc.vector.tensor_copy(osb[:], wop[:])
                        nc.sync.dma_start(
                            out_p[st * 128:(st + 1) * 128, dm * 512:(dm + 1) * 512],
                            osb[:],
                        )

            xt_next = xt0
            for sc_i in range(nsc):
                proj_block(sc_i, xt_next)
                if sc_i == 0:
                    nc.sync.dma_start(
                        wo_sb[:], wo.rearrange("(n p) m -> p n m", p=128))
                if sc_i + 1 < nsc:
                    # prefetch-emit the next chunk's x DMAs so they sit ahead
                    # of the att/wo blocks' output traffic in the queues
                    xt_next = emit_xt(sc_i + 1)
                att_block(sc_i)
                if sc_i >= 1:
                    wo_block(sc_i - 1)
            wo_block(qb_n - 1, final=True)

    nc.compile()
    return nc


_PROGRAM = None


def _get_program():
    global _PROGRAM
    if _PROGRAM is None:
        _PROGRAM = build_program()
    return _PROGRAM


_DEINT = np.concatenate([np.arange(0, DH, 2), np.arange(1, DH, 2)])


def make_in_maps(x, rope_cos, rope_sin, Wq, Wk, Wv, Wo, s=S):
    cosT = rope_cos[:s].T.astype(np.float32)
    sinT = rope_sin[:s].T.astype(np.float32)
    cos2 = np.ascontiguousarray(
        np.stack([cosT, cosT], axis=1).astype(ml_dtypes.bfloat16))
    sin2 = np.ascontiguousarray(
        np.stack([-sinT, sinT], axis=1).astype(ml_dtypes.bfloat16))
    kp = np.arange(128)[:, None]
    cc = np.arange(896)[None, :]
    maskb = (cc >= kp + 384).astype(ml_dtypes.bfloat16)
    in_maps = []
    for c in range(N_CORES):
        b, g = divmod(c, 4)
        xTc = np.ascontiguousarray(x[b].T.astype(ml_dtypes.bfloat16))
        wq_cols = [
            Wq[:, (g * NH + j) * DH:(g * NH + j + 1) * DH][:, _DEINT]
            for j in range(NH)
        ]
        wq_c = np.ascontiguousarray(np.concatenate(wq_cols, axis=1).astype(ml_dtypes.bfloat16))
        wk_c = Wk[:, g * DH:(g + 1) * DH][:, _DEINT]
        wv_c = Wv[:, g * DH:(g + 1) * DH]
        wkv_c = np.ascontiguousarray(
            np.concatenate([wk_c, wv_c], axis=1).astype(ml_dtypes.bfloat16))
        wo_c = np.ascontiguousarray(
            Wo[g * NH * DH:(g + 1) * NH * DH, :].astype(ml_dtypes.bfloat16))
        in_maps.append({
            "xT": xTc, "wq": wq_c, "wkv": wkv_c, "wo": wo_c,
            "cos2": cos2, "sin2": sin2, "maskb": maskb,
        })
    return in_maps


def kernel(x, rope_cos, rope_sin, Wq, Wk, Wv, Wo):
    nc = _get_program()
    in_maps = make_in_maps(x, rope_cos, rope_sin, Wq, Wk, Wv, Wo)
    res = run_bass_kernel_spmd(nc, in_maps, list(range(N_CORES)))
    out = np.zeros((B, S, D), dtype=np.float32)
    for c in range(N_CORES):
        b, g = divmod(c, 4)
        out[b] += res.results[c]["out_p"].astype(np.float32)
    return out


# revision 55
# speedup vs baseline: 1.0729x; 1.0185x over previous
"""GQA attention kernel for Trainium2, sharded over 8 NeuronCores.

Sharding: core c = b*4 + g handles batch b and GQA group g (4 query heads
+ 1 KV head). Wq/Wk/Wv column-sharded per group, Wo row-sharded; the host
sums the 4 per-group partial outputs per batch.

Device layout tricks:
  - x is passed transposed (xT [D, S]) so Q^T/K^T project directly into
    [head_dim, S] layout (head_dim on partitions).
  - V is projected as V^T [dh, S] with N=512 matmuls (cheap), then PE
    transposes produce the [S, dh] tiles PV needs as stationary operand.
  - Q/K head dims are de-interleaved host-side (even dims then odd dims)
    by permuting Wq/Wk columns, making RoPE a half-tile multiply/add.
    Scores are invariant to a shared permutation of Q/K dims.
  - RoPE runs on the DVE in a packed [64, 2, 512] bf16 layout (both
    halves on partitions 0:64, the half-swap becomes a free-dim offset)
    with duplicated cos / sign-folded sin constants: 5 tensor ops per
    chunk, all-SBUF all-bf16 so the DVE's fast mode engages.
  - Attention computes scoresT [key, query] so softmax exp output is
    directly the lhs^T operand ("P^T") for the P@V matmul.
  - 1/sqrt(dh) is folded into the exp activation's scale; the causal mask
    is a 0/1 multiply (GpSimd) on the diagonal-straddling blocks per
    q-block, truncated to the prefix of columns that can contain masked
    entries; the inner loop is software-pipelined by one k-chunk so the
    exp+mask latency hides under the next score matmul.
  - softmax denominator: accumulated on the PE via ones-matmuls into a
    [1,512] PSUM region, then reciprocal_approx_fast + a K=1 ones-matmul
    broadcasts 1/l across partitions for the normalization multiply.
  - Everything on the PE is bf16 (fp32 matmuls run at 1/4 rate).
  - The per-s-chunk projection, the attention q-block it unlocks, and
    the previous q-block's Wo projection are interleaved so every
    engine's latency hides under PE matmul streams.
"""

import sys

if "/opt/trn_rl_repo" not in sys.path:
    sys.path.insert(0, "/opt/trn_rl_repo")

import numpy as np
import ml_dtypes

import concourse.bass as bass
import concourse.bacc as bacc
import concourse.tile as tile
from concourse import mybir
from concourse.bass_utils import run_bass_kernel_spmd
from concourse.masks import make_identity

B = 2
S = 2048
D = 2048
N_HEADS = 16
N_KV = 4
DH = 128
NH = 4  # query heads per core
N_CORES = 8

INV_SQRT_DH = 1.0 / np.sqrt(DH)
F32 = mybir.dt.float32
BF16 = mybir.dt.bfloat16


def build_program(s=S, d=D):
    """Per-core program: 4 query heads + 1 KV head of causal GQA."""
    kc_n = d // 128       # contraction chunks
    sc = 512              # projection s-chunk / q-block
    nsc = s // sc
    qb_n = s // 512

    nc = bacc.Bacc("TRN2", target_bir_lowering=False, debug=False,
                   num_devices=N_CORES)
    xT = nc.declare_dram_parameter("xT", [d, s], BF16, isOutput=False)
    wq = nc.declare_dram_parameter("wq", [d, NH * DH], BF16, isOutput=False)
    wkv = nc.declare_dram_parameter("wkv", [d, 2 * DH], BF16, isOutput=False)
    wo = nc.declare_dram_parameter("wo", [NH * DH, d], BF16, isOutput=False)
    cos2 = nc.declare_dram_parameter("cos2", [64, 2, s], BF16, isOutput=False)
    sin2 = nc.declare_dram_parameter("sin2", [64, 2, s], BF16, isOutput=False)
    maskb = nc.declare_dram_parameter("maskb", [128, 896], BF16, isOutput=False)
    out_p = nc.declare_dram_parameter("out_p", [s, d], BF16, isOutput=True)

    with tile.TileContext(nc) as tc:
        with (
            tc.tile_pool(name="const", bufs=1) as cpool,
            tc.tile_pool(name="xp", bufs=1) as xpool,
            tc.tile_pool(name="act", bufs=1) as apool,
            tc.tile_pool(name="tmp", bufs=1) as tpool,
            tc.tile_pool(name="psum", bufs=1, space="PSUM") as pp,
        ):
            # ---- constants (wo is DMA'd later; it is needed only by the
            #      first wo_block, which runs after the second att block).
            #      wq/wkv are split into chunk DMAs so they spread across
            #      DMA queues and the first projection matmuls start early.
            wq_sb = cpool.tile([128, kc_n, NH * DH], BF16, tag="wq")
            wq_r = wq.rearrange("(n p) m -> p n m", p=128)
            wkv_sb = cpool.tile([128, kc_n, 2 * DH], BF16, tag="wkv")
            wkv_r = wkv.rearrange("(n p) m -> p n m", p=128)
            # The first projection group is K/V: their weights (wkv) and the
            # first x chunks go out first so compute starts ~3us in, with wq
            # following (q0's matmuls interleave per-kc and catch up).
            xt0 = []
            for kc in range(kc_n):
                if kc % 4 == 0:
                    nc.sync.dma_start(wkv_sb[:, kc:kc + 4, :],
                                      wkv_r[:, kc:kc + 4, :])
                t = xpool.tile([128, sc], BF16, tag=f"x{kc}", bufs=2,
                               name=f"xt0_{kc}")
                nc.sync.dma_start(t[:], xT[kc * 128:(kc + 1) * 128, 0:sc])
                xt0.append(t)
            for j in range(8):
                nc.sync.dma_start(wq_sb[:, 2 * j:2 * j + 2, :],
                                  wq_r[:, 2 * j:2 * j + 2, :])
            cos_sb = cpool.tile([64, 2, s], BF16, tag="cos")
            nc.sync.dma_start(cos_sb[:], cos2[:])
            sin_sb = cpool.tile([64, 2, s], BF16, tag="sin")
            nc.sync.dma_start(sin_sb[:], sin2[:])
            mask_sb = cpool.tile([128, 896], BF16, tag="mask")
            nc.sync.dma_start(mask_sb[:], maskb[:])
            ones_col = cpool.tile([128, 1], BF16, tag="ones_col")
            nc.vector.memset(ones_col[:], 1.0)
            ones_row = cpool.tile([1, 128], BF16, tag="ones_row")
            nc.vector.memset(ones_row[:], 1.0)
            ident = cpool.tile([128, 128], BF16, tag="ident")
            make_identity(nc, ident[:])
            wo_sb = cpool.tile([128, NH, d], BF16, tag="wo")

            # ---- persistent activations ----
            ktr = apool.tile([128, s], BF16, tag="ktr")
            qtr = {}   # (h, qb) -> tile
            v_sb = {}  # sc_i -> [128, 4*128] tile (4 key chunks)
            otr = {}   # (h, qb) -> tile

            def rope(dsl, src_psum, sc_i):
                """dsl ([128, sc] slice) = rope(src) with de-interleaved halves.

                src rows 0:64 = even dims (a), 64:128 = odd dims (b).
                re = a*c - b*s -> rows 0:64 ; ro = a*s + b*c -> rows 64:128.
                Packed: braw2 [64, 2, sc] = [a | b] on partitions 0:64; the
                half-swap is a free-dim offset; sin2 carries [-s | +s].
                """
                c = cos_sb[:, :, sc_i * sc:(sc_i + 1) * sc]
                sn = sin_sb[:, :, sc_i * sc:(sc_i + 1) * sc]
                braw2 = tpool.tile([64, 2, sc], BF16, tag="braw", bufs=2)
                nc.scalar.copy(braw2[:, 0, :], src_psum[0:64, :])
                nc.scalar.copy(braw2[:, 1, :], src_psum[64:128, :])
                t1 = tpool.tile([64, 2, sc], BF16, tag="t1", bufs=2)
                t2 = tpool.tile([64, 2, sc], BF16, tag="t2", bufs=2)
                nc.vector.tensor_mul(t1[:], braw2[:], c)
                nc.vector.tensor_mul(t2[:, 0, :], braw2[:, 1, :], sn[:, 0, :])
                nc.vector.tensor_mul(t2[:, 1, :], braw2[:, 0, :], sn[:, 1, :])
                nc.vector.tensor_add(dsl[0:64, :], t1[:, 0, :], t2[:, 0, :])
                nc.vector.tensor_add(dsl[64:128, :], t1[:, 1, :], t2[:, 1, :])

            def emit_xt(sc_i):
                xt = []
                for kc in range(kc_n):
                    t = xpool.tile([128, sc], BF16, tag=f"x{kc}", bufs=2,
                                   name=f"xt{sc_i}_{kc}")
                    nc.sync.dma_start(
                        t[:],
                        xT[kc * 128:(kc + 1) * 128, sc_i * sc:(sc_i + 1) * sc]
                    )
                    xt.append(t)
                return xt

            def proj_block(sc_i, xt):
                # K (4) and V^T (5) first so the att block's first score
                # matmuls never wait on the K rope / V transpose. Groups of 2
                # so the pp PSUM ring needs only 2 banks (freeing one for the
                # double-banked otp).
                for grp in ([4, 5], [0, 1], [2, 3]):
                    psums = {}
                    for hh in grp:
                        psums[hh] = pp.tile([128, sc], F32, tag="pp", bufs=2,
                                            name=f"pj{sc_i}_{hh}")
                    for kc in range(kc_n):
                        for hh in grp:
                            if hh < NH:
                                lhsT = wq_sb[:, kc, hh * DH:(hh + 1) * DH]
                            elif hh == NH:
                                lhsT = wkv_sb[:, kc, 0:DH]
                            else:
                                lhsT = wkv_sb[:, kc, DH:2 * DH]
                            nc.tensor.matmul(
                                psums[hh][:], lhsT, xt[kc][:],
                                start=(kc == 0), stop=(kc == kc_n - 1),
                            )
                    for hh in grp:
                        if hh < NH:
                            qtr[(hh, sc_i)] = apool.tile(
                                [128, 512], BF16, tag=f"qtr{hh}", bufs=4,
                                name=f"qtr{hh}_{sc_i}")
                            rope(qtr[(hh, sc_i)][:, :], psums[hh], sc_i)
                        elif hh == NH:
                            rope(ktr[:, sc_i * sc:(sc_i + 1) * sc], psums[hh], sc_i)
                        else:
                            # V^T [dh, sc] -> bf16 SBUF, then PE-transpose each
                            # 128-wide block into the natural [S, dh] v tiles
                            # (4 transposes packed into one [128,512] PSUM tile).
                            vt_sb = tpool.tile([128, sc], BF16, tag="vt", bufs=2,
                                               name=f"vt{sc_i}")
                            nc.scalar.copy(vt_sb[:], psums[hh][:])
                            vp4 = pp.tile([128, sc], BF16, tag="wp", bufs=2,
                                          name=f"vtp{sc_i}")
                            for stl in range(sc // 128):
                                nc.tensor.transpose(
                                    vp4[:, stl * 128:(stl + 1) * 128],
                                    vt_sb[:, stl * 128:(stl + 1) * 128],
                                    ident[:])
                            v_sb[sc_i] = apool.tile([128, sc], BF16,
                                                    tag=f"v{sc_i}", name=f"v{sc_i}")
                            nc.scalar.copy(v_sb[sc_i][:], vp4[:])

            def att_block(qb):
                # Normalization tail of head h-1 (rlb matmul -> rlb_sb copy
                # -> otr multiply) is deferred until after head h's kc loop:
                # its recip chain completes under a full head of PE matmuls,
                # so the rlb matmul never stalls the PE. otp is double-banked
                # so head h's PV accumulation doesn't wait on head h-1's
                # deferred otr read.
                pend = {}

                def norm_tail(j):
                    rlb = pp.tile([128, 512], F32, tag="lr", bufs=2,
                                  name=f"rlb{j}_{qb}")
                    nc.tensor.matmul(rlb[:], ones_row[:], pend[j][0][:],
                                     start=True, stop=True)
                    rlb_sb = tpool.tile([128, 512], BF16, tag="rlbs", bufs=2,
                                        name=f"rlbs{j}_{qb}")
                    nc.scalar.copy(rlb_sb[:], rlb[:])
                    otr[(j, qb)] = apool.tile([128, 512], BF16, tag=f"otr{j}",
                                              bufs=2, name=f"otr{j}_{qb}")
                    nc.vector.tensor_mul(otr[(j, qb)][:], pend[j][1][:],
                                         rlb_sb[:])

                for h in range(NH):
                    nkc = 4 * (qb + 1)
                    otp = pp.tile([128, 512], F32, tag="ot", bufs=2,
                                  name=f"otp{h}_{qb}")
                    lp = pp.tile([1, 512], F32, tag="lr", bufs=2,
                                 name=f"lp{h}_{qb}")

                    def tail(kc):
                        # PV matmul for chunk kc (issued one chunk late so
                        # exp+mask latency hides under the next score matmul)
                        nc.tensor.matmul(
                            otp[:],
                            v_sb[kc // 4][:, (kc % 4) * 128:(kc % 4 + 1) * 128],
                            pts[kc][:],
                            start=(kc == 0), stop=(kc == nkc - 1),
                        )

                    def tail_l(qd):
                        # denominator ones-matmul per chunk QUAD: adjacent exp
                        # chunks are pre-summed pairwise then quadwise on the
                        # DVE (bf16 fast mode), quartering the PE's lsum
                        # matmul count
                        nc.tensor.matmul(
                            lp[:], ones_col[:], ptquad[qd][:],
                            start=(qd == 0), stop=(qd == nkc // 4 - 1),
                        )

                    pts = {}
                    ptsum = {}
                    ptquad = {}
                    for kc in range(nkc):
                        scp = pp.tile([128, 512], F32, tag="pp", bufs=2,
                                      name=f"scp{h}_{qb}_{kc}")
                        nc.tensor.matmul(
                            scp[:], ktr[:, kc * 128:(kc + 1) * 128],
                            qtr[(h, qb)][:], start=True, stop=True,
                        )
                        pts[kc] = tpool.tile([128, 512], BF16, tag="pt", bufs=4,
                                             name=f"pt{h}_{qb}_{kc}")
                        nc.scalar.activation(
                            pts[kc][:], scp[:], mybir.ActivationFunctionType.Exp,
                            scale=float(INV_SQRT_DH),
                        )
                        if kc >= 4 * qb:  # diagonal-straddling: zero disallowed.
                            # Only cols 0:w can contain masked entries.
                            m = kc - 4 * qb
                            stt = 384 - m * 128
                            w = (m + 1) * 128
                            nc.gpsimd.tensor_mul(pts[kc][:, 0:w],
                                                 pts[kc][:, 0:w],
                                                 mask_sb[:, stt:stt + w])
                        if kc % 2 == 1:
                            ptsum[kc // 2] = tpool.tile(
                                [128, 512], BF16, tag="pts2", bufs=3,
                                name=f"ps{h}_{qb}_{kc // 2}")
                            nc.vector.tensor_add(ptsum[kc // 2][:],
                                                 pts[kc - 1][:], pts[kc][:])
                        if kc % 4 == 3:
                            ptquad[kc // 4] = tpool.tile(
                                [128, 512], BF16, tag="pts4", bufs=2,
                                name=f"pq{h}_{qb}_{kc // 4}")
                            nc.vector.tensor_add(ptquad[kc // 4][:],
                                                 ptsum[kc // 2 - 1][:],
                                                 ptsum[kc // 2][:])
                        if kc > 0:
                            tail(kc - 1)
                        if kc % 4 == 3 and kc // 4 > 0:
                            tail_l(kc // 4 - 1)
                    tail(nkc - 1)
                    tail_l(nkc // 4 - 1)

                    if h > 0:
                        norm_tail(h - 1)
                    rl = tpool.tile([1, 512], F32, tag="rl", bufs=2,
                                    name=f"rl{h}_{qb}")
                    nc.vector.reciprocal_approx_fast(rl[:], lp[:])
                    rl_bf = tpool.tile([1, 512], BF16, tag="rlb16", bufs=2,
                                       name=f"rlbf{h}_{qb}")
                    nc.vector.tensor_copy(rl_bf[:], rl[:])
                    pend[h] = (rl_bf, otp)
                norm_tail(NH - 1)

            def wo_block(qb, final=False):
                for stl in range(4):
                    st = 4 * qb + stl
                    for dm in range(d // 512):
                        wop = pp.tile([128, 512], F32, tag="wp", bufs=2,
                                      name=f"wop{st}_{dm}")
                        for h in range(NH):
                            nc.tensor.matmul(
                                wop[:],
                                otr[(h, qb)][:, stl * 128:(stl + 1) * 128],
                                wo_sb[:, h, dm * 512:(dm + 1) * 512],
                                start=(h == 0), stop=(h == NH - 1),
                            )
                        osb = tpool.tile([128, 512], BF16, tag="osb", bufs=3,
                                         name=f"osb{st}_{dm}")
                        if final and dm % 2 == 1:
                            # the last block has no att block to hide behind;
                            # split the PSUM drains across ACT and DVE
                            nc.scalar.copy(osb[:], wop[:])
                        else:
                            nc.vector.tensor_copy(osb[:], wop[:])
                        nc.sync.dma_start(
                            out_p[st * 128:(st + 1) * 128, dm * 512:(dm + 1) * 512],
                            osb[:],
                        )

            xt_next = xt0
            for sc_i in range(nsc):
                proj_block(sc_i, xt_next)
                if sc_i == 0:
                    nc.sync.dma_start(
                        wo_sb[:], wo.rearrange("(n p) m -> p n m", p=128))
                if sc_i + 1 < nsc:
                    # prefetch-emit the next chunk's x DMAs so they sit ahead
                    # of the att/wo blocks' output traffic in the queues
                    xt_next = emit_xt(sc_i + 1)
                att_block(sc_i)
                if sc_i >= 1:
                    wo_block(sc_i - 1)
            wo_block(qb_n - 1, final=True)

    nc.compile()
    return nc


_PROGRAM = None


def _get_program():
    global _PROGRAM
    if _PROGRAM is None:
        _PROGRAM = build_program()
    return _PROGRAM


_DEINT = np.concatenate([np.arange(0, DH, 2), np.arange(1, DH, 2)])


def make_in_maps(x, rope_cos, rope_sin, Wq, Wk, Wv, Wo, s=S):
    cosT = rope_cos[:s].T.astype(np.float32)
    sinT = rope_sin[:s].T.astype(np.float32)
    cos2 = np.ascontiguousarray(
        np.stack([cosT, cosT], axis=1).astype(ml_dtypes.bfloat16))
    sin2 = np.ascontiguousarray(
        np.stack([-sinT, sinT], axis=1).astype(ml_dtypes.bfloat16))
    kp = np.arange(128)[:, None]
    cc = np.arange(896)[None, :]
    maskb = (cc >= kp + 384).astype(ml_dtypes.bfloat16)
    in_maps = []
    for c in range(N_CORES):
        b, g = divmod(c, 4)
        xTc = np.ascontiguousarray(x[b].T.astype(ml_dtypes.bfloat16))
        wq_cols = [
            Wq[:, (g * NH + j) * DH:(g * NH + j + 1) * DH][:, _DEINT]
            for j in range(NH)
        ]
        wq_c = np.ascontiguousarray(np.concatenate(wq_cols, axis=1).astype(ml_dtypes.bfloat16))
        wk_c = Wk[:, g * DH:(g + 1) * DH][:, _DEINT]
        wv_c = Wv[:, g * DH:(g + 1) * DH]
        wkv_c = np.ascontiguousarray(
            np.concatenate([wk_c, wv_c], axis=1).astype(ml_dtypes.bfloat16))
        wo_c = np.ascontiguousarray(
            Wo[g * NH * DH:(g + 1) * NH * DH, :].astype(ml_dtypes.bfloat16))
        in_maps.append({
            "xT": xTc, "wq": wq_c, "wkv": wkv_c, "wo": wo_c,
            "cos2": cos2, "sin2": sin2, "maskb": maskb,
        })
    return in_maps


def kernel(x, rope_cos, rope_sin, Wq, Wk, Wv, Wo):
    nc = _get_program()
    in_maps = make_in_maps(x, rope_cos, rope_sin, Wq, Wk, Wv, Wo)
    res = run_bass_kernel_spmd(nc, in_maps, list(range(N_CORES)))
    out = np.zeros((B, S, D), dtype=np.float32)
    for c in range(N_CORES):
        b, g = divmod(c, 4)
        out[b] += res.results[c]["out_p"].astype(np.float32)
    return out
